# revision 10
# baseline (speedup 1.0000x reference)
"""Trainium2 Bass kernel for BlockwiseEarlyExitMamba (v2).

Model: packet embedder -> 4 Mamba blocks (d_model=256, d_inner=512,
d_state=16, dt_rank=16, d_conv=4) -> LayerNorm chain -> early-exit MLP
classifier that reads ONLY position min(32, L)-1 = 31.

Every op in the network is causal, so the [B, 2] output depends only on
x[:, :32, :]; we compute 32 timesteps instead of 1024 (exact).

Sharding: data-parallel over batch, 2 samples/core, weights replicated.

v2 changes vs v1 (222us):
 - all per-layer weights packed host-side into ONE bf16 blob + ONE small
   f32 blob per layer (12 DMA issues total vs ~59 at ~640ns queue time
   each); blobs spread across tensor/scalar/sync queues.
 - in_proj emits directly in channel-major layout ([d partitions, (b t)])
   with the weight chunk as the stationary operand: kills 8 PE transposes
   + 6 scalar copies per layer.
 - dt path: W_dtfull = dt_w @ x_proj_w[:16] precomputed on host (skips
   the serial x_proj->dt matmul hop); dt_b pre-filled into PSUM so
   softplus is 2 wide ACTs instead of 5.
 - B/C broadcast: 2 DRAM writes + ONE contiguous stride-0 read.
 - h*C: ONE contiguous GpSimd tensor_tensor over [128, 4096] (v1: 4
   scattered ones at ~4x the per-element cost); n-reduction via an
   in-place pairwise tree (contiguous reads) instead of 4 reduces.
 - scnb (dt*x*B) built with 2 wide TTs (per sample) instead of 4.
"""

import os
import sys

import numpy as np

for _p in ("/root/.axon_site/_ro/trn_rl_repo", "/opt/trn_rl_repo"):
    if os.path.isdir(_p) and _p not in sys.path:
        sys.path.insert(0, _p)

import concourse.bacc as bacc
import concourse.bass as bass
import concourse.mybir as mybir
import concourse.tile as tile
from concourse.bass_utils import run_bass_kernel_spmd

F32 = mybir.dt.float32
BF16 = mybir.dt.bfloat16
AF = mybir.ActivationFunctionType
ALU = mybir.AluOpType

# Pin every activation func this kernel uses to ONE ACT table set, so the
# table-load placement pass emits a single load instead of thrashing.
_ACT_SET = "natural_log_exp_and_others"
_MY_FUNCS = {AF.Exp, AF.Ln, AF.Relu, AF.Square, AF.Identity, AF.Copy}
_orig_get_tables = bacc.get_activation_tables


def _pinned_tables(arch):
    tabs = _orig_get_tables(arch)
    assert _MY_FUNCS <= tabs[_ACT_SET]
    return {name: (funcs if name == _ACT_SET else funcs - _MY_FUNCS)
            for name, funcs in tabs.items()}


bacc.get_activation_tables = _pinned_tables

# Model dims
D_MODEL = 256
D_INNER = 512
D_STATE = 16
D_CONV = 4
DT_RANK = 16
N_LAYERS = 4
BATCH = 16
SEQLEN = 1024
T = 32          # effective timesteps (causal truncation)
N_CORES = 8
B_LOC = BATCH // N_CORES   # 2 samples per core
TOK = B_LOC * T            # 64 tokens per core
NJ = D_INNER // 128        # 4 channel chunks
DM_ROWS = 256 + 1 + 64 + 1 + 2 + 1  # 325 design-matrix rows
SEG = T + 3                # 35: one conv segment incl. 3-col zero gap

# bf16 blob column layout (per layer): [128, WB_COLS]
#   WINT  + (k*8+j)*128 : in_proj stationary chunk; j 0..3 -> x c=j,
#                         4..7 -> z c=j-4; k = d_model chunk
#   WDTF  + (k2*4+c)*128: Wdtf^T chunk, Wdtf = dt_w @ x_proj_w[:16]
#   WOUT  + c*256       : out_proj^T chunk
#   WXBC  + k2*32       : x_proj B/C rows ^T chunk
WINT, WDTF, WOUT, WXBC = 0, 2048, 4096, 5120
WB_COLS = 5248
# f32 blob = smalls [128, 108]:
#   0:32 conv_w (c,b,k), 32:36 conv_b, 36:40 dt_b, 40:104 A, 104:108 D
FB_COLS = 108


def _build_program(a_vals):
    nc = bacc.Bacc(None, target_bir_lowering=False, debug=False)

    # ---------------- DRAM I/O ----------------
    x_d = nc.dram_tensor("x_local", [TOK, 5], F32, kind="ExternalInput")
    embw_d = nc.dram_tensor("embw", [128, 3 * D_MODEL], F32, kind="ExternalInput")
    wblob_d = nc.dram_tensor("wblob", [N_LAYERS, 128, WB_COLS], BF16,
                             kind="ExternalInput")
    fblob_d = nc.dram_tensor("fblob", [N_LAYERS, 128, FB_COLS], F32,
                             kind="ExternalInput")
    # cls blob: [128, 256 w1t (2x128) | 1 b1 | 2 w2t | 1 b2(rows 0:2)]
    cblob_d = nc.dram_tensor("cblob", [128, 260], F32, kind="ExternalInput")
    out_d = nc.dram_tensor("out", [2, B_LOC], F32, kind="ExternalOutput")

    # B/C scratch, laid out [s][b][n][t] so the broadcast read is contiguous
    bc_scr = nc.dram_tensor("bc_scr", [2 * B_LOC * D_STATE * T], F32)
    HALF = B_LOC * D_STATE * T  # 1024

    with tile.TileContext(nc) as tc:
        with (
            tc.tile_pool(name="const", bufs=1) as cp,
            tc.tile_pool(name="wpool", bufs=1) as wp,
            tc.tile_pool(name="work", bufs=1) as rp,
            tc.tile_pool(name="scan", bufs=1) as sp,
            tc.tile_pool(name="psmm", bufs=2, space="PSUM") as pmm,
            tc.tile_pool(name="pstr", bufs=2, space="PSUM") as ptr,
            tc.tile_pool(name="psxz", bufs=1, space="PSUM") as pxz,
        ):
            # -------- input + weight DMAs (few, spread over queues) --------
            xq = rp.tile([TOK, 5], F32, name="xq")
            nc.sync.dma_start(xq[:], x_d[:])
            embw_sb = wp.tile([128, 3 * D_MODEL], F32, name="embw")
            nc.sync.dma_start(embw_sb[:], embw_d[:])

            wblob_sb, fblob_sb = [], []
            for l in range(N_LAYERS):
                wt = wp.tile([128, WB_COLS], BF16, name=f"wblob{l}")
                (nc.scalar, nc.gpsimd, nc.scalar, nc.gpsimd)[l].dma_start(
                    wt[:], wblob_d[l])
                wblob_sb.append(wt)
                ft = wp.tile([128, FB_COLS], F32, name=f"fblob{l}")
                nc.sync.dma_start(ft[:], fblob_d[l])
                fblob_sb.append(ft)
            cblob_sb = wp.tile([128, 260], F32, name="cblob")
            nc.sync.dma_start(cblob_sb[:], cblob_d[:])

            # ---------------- constants ----------------
            ident = cp.tile([128, 128], F32, name="ident")
            nc.gpsimd.memset(ident[:], 0.0)
            nc.gpsimd.affine_select(
                out=ident[:], in_=ident[:], compare_op=ALU.not_equal,
                fill=1.0, base=0, pattern=[[-1, 128]], channel_multiplier=1)
            iota257 = cp.tile([TOK, 257], F32, name="iota257")
            nc.gpsimd.iota(iota257[:], pattern=[[1, 257]], base=0,
                           channel_multiplier=0,
                           allow_small_or_imprecise_dtypes=True)
            eps_t = cp.tile([128, 1], F32, name="eps_t")
            nc.vector.memset(eps_t[:], 1e-5)

            # ---------------- embedder ----------------
            # One-hot of int(clip(x)) as a difference of >= comparisons.
            dm = rp.tile([TOK, DM_ROWS], F32, name="dm")
            ge_p = rp.tile([TOK, 257], F32, name="ge_p")
            nc.vector.tensor_tensor(
                ge_p[:], xq[:, 0:1].broadcast_to([TOK, 257]), iota257[:],
                op=ALU.is_ge)
            nc.vector.tensor_sub(dm[:, 0:256], ge_p[:, 0:256], ge_p[:, 1:257])
            ge_f = rp.tile([TOK, 65], F32, name="ge_f")
            nc.vector.tensor_tensor(
                ge_f[:], xq[:, 2:3].broadcast_to([TOK, 65]), iota257[:, 0:65],
                op=ALU.is_ge)
            nc.vector.tensor_sub(dm[:, 257:321], ge_f[:, 0:64], ge_f[:, 1:65])
            ge_d = rp.tile([TOK, 3], F32, name="ge_d")
            nc.vector.tensor_tensor(
                ge_d[:], xq[:, 4:5].broadcast_to([TOK, 3]), iota257[:, 0:3],
                op=ALU.is_ge)
            nc.vector.tensor_sub(dm[:, 322:324], ge_d[:, 0:2], ge_d[:, 1:3])
            dmcols = bass.AP(dm[:].tensor, dm[:, 256].offset,
                             [dm[:].ap[0], [65, 2]])
            xqcols = bass.AP(xq[:].tensor, xq[:, 1].offset,
                             [xq[:].ap[0], [2, 2]])
            nc.scalar.copy(dmcols, xqcols)
            nc.vector.memset(dm[:, 324:325], 1.0)

            feat_ps = pmm.tile([TOK, D_MODEL], F32, name="feat_ps", tag="mm")
            for c, (r0, r1) in enumerate(((0, 128), (128, 256), (256, DM_ROWS))):
                w = r1 - r0
                tp = ptr.tile([128, TOK], F32, name=f"dmt_ps{c}", tag="tr")
                nc.tensor.transpose(tp[:w, :], dm[:, r0:r1], ident[:TOK, :TOK])
                dmt = rp.tile([128, TOK], F32, name=f"dmt{c}", tag="dmt")
                nc.scalar.copy(dmt[:w, :], tp[:w, :])
                nc.tensor.matmul(feat_ps[:], dmt[:w, :],
                                 embw_sb[:w, c * D_MODEL:(c + 1) * D_MODEL],
                                 start=(c == 0), stop=(c == 2))

            def layer_norm(src_ap, dst):
                """dst = LN(src) over free dim (256), no affine (g=1, b=0)."""
                nsum = rp.tile([TOK, 1], F32, name="nsum", tag="lnstat")
                nc.vector.tensor_reduce(nsum[:], src_ap, axis=mybir.AxisListType.X,
                                        op=ALU.add, negate=True)
                nmean = rp.tile([TOK, 1], F32, name="nmean", tag="lnstat2")
                nc.scalar.mul(nmean[:], nsum[:], 1.0 / D_MODEL)
                cen = rp.tile([TOK, D_MODEL], F32, name="cen", tag="lncen")
                nc.vector.tensor_scalar_add(cen[:], src_ap, nmean[:])
                sq = rp.tile([TOK, D_MODEL], F32, name="sq", tag="lnsq")
                vsum = rp.tile([TOK, 1], F32, name="vsum", tag="lnstat3")
                nc.scalar.activation(sq[:], cen[:], AF.Square, accum_out=vsum[:])
                lnv = rp.tile([TOK, 1], F32, name="lnv", tag="lnstat4")
                nc.scalar.activation(lnv[:], vsum[:], AF.Ln,
                                     bias=eps_t[:TOK, :], scale=1.0 / D_MODEL)
                rstd = rp.tile([TOK, 1], F32, name="rstd", tag="lnstat5")
                nc.scalar.activation(rstd[:], lnv[:], AF.Exp, scale=-0.5)
                nc.vector.tensor_scalar_mul(dst, cen[:], rstd[:])

            feat = rp.tile([TOK, D_MODEL], F32, name="feat_init")
            layer_norm(feat_ps[:], feat[:])

            # ---------------- Mamba layers ----------------
            # conv scratch with zero gaps (zeroed once, stays zero)
            xpad = rp.tile([128, NJ * B_LOC * SEG], F32, name="xpad")
            gaps = bass.AP(xpad[:].tensor, xpad[:].offset,
                           [xpad[:].ap[0], [SEG, NJ * B_LOC], [1, 3]])
            nc.vector.memset(gaps, 0.0)
            for l in range(N_LAYERS):
                wb = wblob_sb[l]
                fb = fblob_sb[l]

                # dt_b pre-fill of the dtpre PSUM accumulator (c varies,
                # broadcast over (b t)); matmuls below use start=False.
                dtpre_ps = pmm.tile([128, NJ * TOK], F32, name=f"dtpre{l}",
                                    tag="mm")
                dtb_src = bass.AP(fb[:].tensor, fb[:, 36].offset,
                                  [fb[:].ap[0], [1, NJ], [0, TOK]])
                dtb_dst = bass.AP(dtpre_ps[:].tensor, dtpre_ps[:].offset,
                                  [dtpre_ps[:].ap[0], [TOK, NJ], [1, TOK]])
                nc.vector.tensor_scalar_add(dtb_dst, dtb_src, 0.0)

                # featT [256, TOK] as two 128-row chunks, bf16
                featT = rp.tile([128, 2 * TOK], BF16, name=f"featT{l}",
                                tag="featT")
                for c in range(2):
                    tp = ptr.tile([128, TOK], F32, name=f"ftp{l}_{c}", tag="tr")
                    nc.tensor.transpose(tp[:], feat[:, c * 128:(c + 1) * 128],
                                        ident[:TOK, :TOK])
                    nc.scalar.copy(featT[:, c * TOK:(c + 1) * TOK], tp[:])

                # in_proj directly into channel-major layout:
                # xz[j-chunk, (b t)] in PSUM; j 0..3 -> x c=j, 4..7 -> z.
                xz_ps = pxz.tile([128, 8 * TOK], F32, name=f"xz{l}", tag="xz")
                for j in range(8):
                    for k in range(2):
                        nc.tensor.matmul(
                            xz_ps[:, j * TOK:(j + 1) * TOK],
                            wb[:, WINT + (k * 8 + j) * 128:
                               WINT + (k * 8 + j + 1) * 128],
                            featT[:, k * TOK:(k + 1) * TOK],
                            start=(k == 0), stop=(k == 1))

                # conv: one wide PSUM->zero-gap-SBUF copy, then tap-product
                # + tap-reduce + bias add.
                cpsrc = bass.AP(xz_ps[:].tensor, xz_ps[:].offset,
                                [xz_ps[:].ap[0], [T, NJ * B_LOC], [1, T]])
                cpdst = bass.AP(xpad[:].tensor, xpad[:, 3].offset,
                                [xpad[:].ap[0], [SEG, NJ * B_LOC], [1, T]])
                nc.scalar.copy(cpdst, cpsrc)
                cprod = rp.tile([128, NJ * B_LOC, T, D_CONV], F32,
                                name=f"cprod{l}", tag="cprod")
                in0 = bass.AP(xpad[:].tensor, xpad[:].offset,
                              [xpad[:].ap[0], [SEG, NJ * B_LOC], [1, T],
                               [1, D_CONV]])
                in1 = bass.AP(fb[:].tensor, fb[:].offset,
                              [fb[:].ap[0], [D_CONV, NJ * B_LOC], [0, T],
                               [1, D_CONV]])
                nc.vector.tensor_tensor(cprod[:], in0, in1, op=ALU.mult)
                vpre = rp.tile([128, NJ, B_LOC, T], F32, name=f"vpre{l}",
                               tag="vpre")
                nc.vector.tensor_reduce(
                    vpre[:].rearrange("p a b t -> p (a b) t"), cprod[:],
                    axis=mybir.AxisListType.X, op=ALU.add)
                cb_ap = bass.AP(fb[:].tensor, fb[:, 32].offset,
                                [fb[:].ap[0], [1, NJ], [0, B_LOC], [0, T]])
                nc.vector.tensor_add(vpre[:], vpre[:], cb_ap)

                # silu(v) = v * sigmoid(v); sigmoid via exp/ln chain.
                # xcall comes out in bf16 (it is a matmul operand below).
                vflat = vpre[:].rearrange("p a b t -> p (a b t)")
                sg = rp.tile([128, NJ * B_LOC * T], F32, name=f"sg{l}", tag="sg")
                nc.scalar.activation(sg[:], vflat, AF.Exp, scale=-1.0)
                nc.scalar.activation(sg[:], sg[:], AF.Ln, bias=1.0)
                nc.scalar.activation(sg[:], sg[:], AF.Exp, scale=-1.0)
                xcall = rp.tile([128, NJ, B_LOC, T], BF16, name=f"xcall{l}",
                                tag="xcall")
                nc.vector.tensor_mul(
                    xcall[:].rearrange("p a b t -> p (a b t)"), vflat, sg[:])

                # x_proj B/C rows + dt_pre, straight from xcall chunks.
                dbl_ps = ptr.tile([2 * D_STATE, TOK], F32, name=f"dbl{l}",
                                  tag="tr")
                for k2 in range(NJ):
                    nc.tensor.matmul(
                        dbl_ps[:],
                        wb[:, WXBC + k2 * 32:WXBC + (k2 + 1) * 32],
                        xcall[:, k2].rearrange("p b t -> p (b t)"),
                        start=(k2 == 0), stop=(k2 == NJ - 1))
                for c in range(NJ):
                    for k2 in range(NJ):
                        nc.tensor.matmul(
                            dtpre_ps[:, c * TOK:(c + 1) * TOK],
                            wb[:, WDTF + (k2 * 4 + c) * 128:
                               WDTF + (k2 * 4 + c + 1) * 128],
                            xcall[:, k2].rearrange("p b t -> p (b t)"),
                            start=False, stop=(k2 == NJ - 1),
                            skip_group_check=True)

                # B/C -> DRAM in [s][b][n][t] order, then ONE stride-0
                # broadcast read across all 128 partitions.
                dbl_sb = rp.tile([2 * D_STATE, TOK], F32, name=f"dblsb{l}",
                                 tag="dblsb")
                nc.scalar.copy(dbl_sb[:], dbl_ps[:])
                for s in range(2):
                    src = dbl_sb[s * D_STATE:(s + 1) * D_STATE, :]
                    dst = bass.AP(bc_scr[:].tensor, s * HALF,
                                  [[T, D_STATE], [D_STATE * T, B_LOC], [1, T]])
                    nc.gpsimd.dma_start(dst, src)
                bcrep = rp.tile([128, 2 * HALF], F32, name=f"bcrep{l}",
                                tag="bcrep")
                nc.gpsimd.dma_start(
                    bcrep[:],
                    bass.AP(bc_scr[:].tensor, 0, [[0, 128], [1, 2 * HALF]]))

                # softplus(dtpre) = ln(1 + exp(dtpre)) -- 2 wide ACTs
                # (bias is already in the PSUM accumulator)
                dtall = rp.tile([128, NJ, B_LOC, T], F32, name=f"dtall{l}",
                                tag="dtall")
                dtflat = dtall[:].rearrange("p a b t -> p (a b t)")
                nc.scalar.activation(dtflat, dtpre_ps[:], AF.Exp, scale=1.0)
                nc.scalar.activation(dtflat, dtflat, AF.Ln, bias=1.0)

                # dA = exp(dt * A) with dA[t=0 of each (c,b,n) segment] = 0
                scna = sp.tile([128, NJ, B_LOC, D_STATE, T], F32,
                               name=f"scna{l}", tag="scna")
                t0 = bass.AP(scna[:].tensor, scna[:].offset,
                             [scna[:].ap[0], [B_LOC * D_STATE * T, NJ],
                              [T, B_LOC * D_STATE], [1, 1]])
                if a_vals is not None:
                    nc.vector.memset(t0, 0.0)
                    for n in range(D_STATE):
                        src = bass.AP(
                            dtall[:].tensor, dtall[:, 0, 0, 1].offset,
                            [dtall[:].ap[0], [B_LOC * T, NJ], [T, B_LOC],
                             [1, T - 1]])
                        dst = bass.AP(
                            scna[:].tensor, scna[:, 0, 0, n, 1].offset,
                            [scna[:].ap[0], [B_LOC * D_STATE * T, NJ],
                             [D_STATE * T, B_LOC], [1, T - 1]])
                        nc.scalar.activation(dst, src, AF.Exp,
                                             scale=float(a_vals[l][n]))
                else:
                    for c in range(NJ):
                        in0g = bass.AP(
                            dtall[:].tensor, dtall[:, c, 0, 0].offset,
                            [dtall[:].ap[0], [T, B_LOC], [0, D_STATE], [1, T]])
                        in1g = bass.AP(
                            fb[:].tensor, fb[:, 40 + c * D_STATE].offset,
                            [fb[:].ap[0], [0, B_LOC], [1, D_STATE], [0, T]])
                        nc.vector.tensor_tensor(
                            scna[:, c], in0g, in1g, op=ALU.mult)
                    body = bass.AP(
                        scna[:].tensor, scna[:, 0, 0, 0, 1].offset,
                        [scna[:].ap[0], [T, NJ * B_LOC * D_STATE], [1, T - 1]])
                    nc.scalar.activation(body, body, AF.Exp)
                    nc.vector.memset(t0, 0.0)

                # dtx = dt * xc
                dtx = rp.tile([128, NJ, B_LOC, T], F32, name=f"dtx{l}",
                              tag="dtx")
                nc.vector.tensor_mul(
                    dtx[:].rearrange("p a b t -> p (a b t)"), dtflat,
                    xcall[:].rearrange("p a b t -> p (a b t)"))

                # scnb = dtx (bcast n) * B_rep -- one wide TT per sample
                scnb = sp.tile([128, NJ, B_LOC, D_STATE, T], F32,
                               name=f"scnb{l}", tag="scnb")
                for b in range(B_LOC):
                    in0b = bass.AP(dtx[:].tensor, dtx[:, 0, b, 0].offset,
                                   [dtx[:].ap[0], [B_LOC * T, NJ],
                                    [0, D_STATE], [1, T]])
                    in1b = bass.AP(bcrep[:].tensor,
                                   bcrep[:, b * D_STATE * T].offset,
                                   [bcrep[:].ap[0], [0, NJ], [T, D_STATE],
                                    [1, T]])
                    outb = bass.AP(scnb[:].tensor, scnb[:, 0, b, 0, 0].offset,
                                   [scnb[:].ap[0],
                                    [B_LOC * D_STATE * T, NJ],
                                    [T, D_STATE], [1, T]])
                    nc.vector.tensor_tensor(outb, in0b, in1b, op=ALU.mult)

                # sigmoid(z) for the output gate, straight off the z PSUM
                # half; emitted after the dA ACTs so it fills the scalar
                # queue during the scan.
                zraw = bass.AP(xz_ps[:].tensor, xz_ps[:, 4 * TOK].offset,
                               [xz_ps[:].ap[0], [1, NJ * B_LOC * T]])
                zsig = rp.tile([128, NJ * B_LOC * T], F32, name=f"zsig{l}",
                               tag="zsig")
                nc.scalar.activation(zsig[:], zraw, AF.Exp, scale=-1.0)
                nc.scalar.activation(zsig[:], zsig[:], AF.Ln, bias=1.0)
                nc.scalar.activation(zsig[:], zsig[:], AF.Exp, scale=-1.0)
                nc.vector.tensor_mul(zsig[:], zsig[:], zraw)

                # THE scan: h[t] = dA[t] * h[t-1] + dBx[t] along free dim
                hh = sp.tile([128, NJ, B_LOC, D_STATE, T], F32,
                             name=f"hh{l}", tag="hh")
                for c in range(NJ):
                    nc.vector.tensor_tensor_scan(
                        hh[:, c].rearrange("p b n t -> p (b n t)"),
                        scna[:, c].rearrange("p b n t -> p (b n t)"),
                        scnb[:, c].rearrange("p b n t -> p (b n t)"),
                        initial=0.0, op0=ALU.mult, op1=ALU.add)

                # hc = h * C_rep: ONE contiguous GpSimd TT over [128, 4096]
                hc = sp.tile([128, NJ, B_LOC, D_STATE, T], F32,
                             name=f"hc{l}", tag="hc")
                NT = D_STATE * T
                hh_ap = bass.AP(hh[:].tensor, hh[:].offset,
                                [hh[:].ap[0], [B_LOC * NT, NJ], [NT, B_LOC],
                                 [1, NT]])
                cr_ap = bass.AP(bcrep[:].tensor, bcrep[:, HALF].offset,
                                [bcrep[:].ap[0], [0, NJ], [NT, B_LOC],
                                 [1, NT]])
                hc_ap = bass.AP(hc[:].tensor, hc[:].offset,
                                [hc[:].ap[0], [B_LOC * NT, NJ], [NT, B_LOC],
                                 [1, NT]])
                nc.gpsimd.tensor_tensor(hc_ap, hh_ap, cr_ap, op=ALU.mult)

                # n-reduction: in-place pairwise tree over n (contiguous
                # (n t) runs); result lands in the n=0 slice of hc.
                h = D_STATE
                while h > 1:
                    h //= 2
                    lo = bass.AP(hc[:].tensor, hc[:].offset,
                                 [hc[:].ap[0], [NT, NJ * B_LOC], [T, h],
                                  [1, T]])
                    hi = bass.AP(hc[:].tensor, hc[:, 0, 0, h, 0].offset,
                                 [hc[:].ap[0], [NT, NJ * B_LOC], [T, h],
                                  [1, T]])
                    nc.vector.tensor_add(lo, lo, hi)
                ys_ap = bass.AP(hc[:].tensor, hc[:].offset,
                                [hc[:].ap[0], [NT, NJ * B_LOC], [1, T]])

                # y = (ys + D * xc) * z * sigmoid(z)
                yg = rp.tile([128, NJ, B_LOC, T], F32, name=f"yg{l}", tag="yg")
                d_ap = bass.AP(fb[:].tensor, fb[:, 104].offset,
                               [fb[:].ap[0], [1, NJ], [0, B_LOC], [0, T]])
                nc.vector.tensor_mul(yg[:], xcall[:], d_ap)
                ygf = bass.AP(yg[:].tensor, yg[:].offset,
                              [yg[:].ap[0], [T, NJ * B_LOC], [1, T]])
                nc.vector.tensor_add(ygf, ygf, ys_ap)
                ygr = rp.tile([128, NJ, B_LOC, T], BF16, name=f"ygr{l}",
                              tag="ygr")
                nc.vector.tensor_mul(
                    ygr[:].rearrange("p a b t -> p (a b t)"),
                    yg[:].rearrange("p a b t -> p (a b t)"), zsig[:])

                # out_proj + residual + LN
                yout_ps = pmm.tile([TOK, D_MODEL], F32, name=f"yout{l}",
                                   tag="mm2")
                for c in range(NJ):
                    nc.tensor.matmul(
                        yout_ps[:], ygr[:, c].rearrange("p b t -> p (b t)"),
                        wb[:, WOUT + c * D_MODEL:WOUT + (c + 1) * D_MODEL],
                        start=(c == 0), stop=(c == NJ - 1))
                fsum = rp.tile([TOK, D_MODEL], F32, name=f"fsum{l}", tag="fsum")
                nc.vector.tensor_add(fsum[:], yout_ps[:], feat[:])
                feat = rp.tile([TOK, D_MODEL], F32, name=f"feat{l}",
                               tag="featv2")
                layer_norm(fsum[:], feat[:])

            # ---------------- classifier (token t=31 per sample) ----------
            cls_in = rp.tile([B_LOC, D_MODEL], F32, name="cls_in")
            for b in range(B_LOC):
                r = b * T + (T - 1)
                nc.sync.dma_start(cls_in[b:b + 1, :], feat[r:r + 1, :])
            clsT = rp.tile([128, 2 * B_LOC], F32, name="clsT")
            for c in range(2):
                tp = ptr.tile([128, B_LOC], F32, name=f"clsT_ps{c}", tag="tr")
                nc.tensor.transpose(tp[:], cls_in[:, c * 128:(c + 1) * 128],
                                    ident[:B_LOC, :B_LOC])
                nc.scalar.copy(clsT[:, c * B_LOC:(c + 1) * B_LOC], tp[:])
            q1_ps = pmm.tile([128, B_LOC], F32, name="q1_ps", tag="mm")
            for c in range(2):
                nc.tensor.matmul(q1_ps[:], cblob_sb[:, c * 128:(c + 1) * 128],
                                 clsT[:, c * B_LOC:(c + 1) * B_LOC],
                                 start=(c == 0), stop=(c == 1))
            r1 = rp.tile([128, B_LOC], F32, name="r1")
            nc.scalar.activation(r1[:], q1_ps[:], AF.Relu,
                                 bias=cblob_sb[:, 256:257], scale=1.0)
            o_ps = pmm.tile([2, B_LOC], F32, name="o_ps", tag="mm2")
            nc.tensor.matmul(o_ps[:], cblob_sb[:, 257:259], r1[:],
                             start=True, stop=True)
            out_sb = rp.tile([2, B_LOC], F32, name="out_sb")
            nc.scalar.activation(out_sb[:], o_ps[:], AF.Identity,
                                 bias=cblob_sb[0:2, 259:260], scale=1.0)
            nc.sync.dma_start(out_d[:], out_sb[:])

    nc.finalize()
    return nc


def _prep_host(inputs):
    """Host-side weight preprocessing (pure reshaping/merging, exact math)."""
    import ml_dtypes

    g = lambda k: np.asarray(inputs[k], dtype=np.float32)

    fusion_w = g("fusion_w")          # [256, 136]
    wf_proto = fusion_w[:, 0:32]
    wf_len = fusion_w[:, 32:64]
    wf_flags = fusion_w[:, 64:96]
    wf_iat = fusion_w[:, 96:128]
    wf_dir = fusion_w[:, 128:136]

    embw = np.zeros((DM_ROWS, D_MODEL), np.float32)
    embw[0:256] = g("emb_proto") @ wf_proto.T
    embw[256] = wf_len @ g("proj_len_w")[:, 0]
    embw[257:321] = g("emb_flags") @ wf_flags.T
    embw[321] = wf_iat @ g("proj_iat_w")[:, 0]
    embw[322:324] = g("emb_dir") @ wf_dir.T
    embw[324] = (g("fusion_b") + wf_len @ g("proj_len_b")
                 + wf_iat @ g("proj_iat_b"))
    embw_p = np.zeros((128, 3 * D_MODEL), np.float32)
    for c, (r0, r1) in enumerate(((0, 128), (128, 256), (256, DM_ROWS))):
        embw_p[:r1 - r0, c * D_MODEL:(c + 1) * D_MODEL] = embw[r0:r1]

    A = -np.exp(g("A_log"))           # [L, 512, 16]
    if bool(np.all(A == A[:, :1, :])):
        a_vals = tuple(tuple(float(v) for v in A[l, 0]) for l in range(N_LAYERS))
    else:
        a_vals = None

    wblob = np.zeros((N_LAYERS, 128, WB_COLS), ml_dtypes.bfloat16)
    fblob = np.zeros((N_LAYERS, 128, FB_COLS), np.float32)
    for l in range(N_LAYERS):
        wint = g("in_proj_w")[l].T            # [256, 1024]
        for k in range(2):
            for j in range(8):
                wblob[l, :, WINT + (k * 8 + j) * 128:
                      WINT + (k * 8 + j + 1) * 128] = \
                    wint[k * 128:(k + 1) * 128, j * 128:(j + 1) * 128]
        wdtf = (g("dt_w")[l] @ g("x_proj_w")[l][:DT_RANK, :]).T  # [din, dout]
        for k2 in range(NJ):
            for c in range(NJ):
                wblob[l, :, WDTF + (k2 * 4 + c) * 128:
                      WDTF + (k2 * 4 + c + 1) * 128] = \
                    wdtf[k2 * 128:(k2 + 1) * 128, c * 128:(c + 1) * 128]
        wout = g("out_proj_w")[l].T           # [512, 256]
        for c in range(NJ):
            wblob[l, :, WOUT + c * D_MODEL:WOUT + (c + 1) * D_MODEL] = \
                wout[c * 128:(c + 1) * 128]
        wxbc = g("x_proj_w")[l][DT_RANK:, :].T  # [512, 32]
        for k2 in range(NJ):
            wblob[l, :, WXBC + k2 * 32:WXBC + (k2 + 1) * 32] = \
                wxbc[k2 * 128:(k2 + 1) * 128]

        cw = g("conv_w")[l].reshape(NJ, 128, D_CONV)          # [j, p, k]
        cwp = np.transpose(cw, (1, 0, 2))                     # [p, j, k]
        fblob[l, :, 0:32] = np.repeat(cwp, B_LOC, axis=1).reshape(128, 32)
        fblob[l, :, 32:36] = g("conv_b")[l].reshape(NJ, 128).T
        fblob[l, :, 36:40] = g("dt_b")[l].reshape(NJ, 128).T
        Aj = A[l].reshape(NJ, 128, D_STATE)                   # [j, p, n]
        fblob[l, :, 40:104] = np.transpose(Aj, (1, 0, 2)).reshape(128, 64)
        fblob[l, :, 104:108] = g("D_param")[l].reshape(NJ, 128).T

    cblob = np.zeros((128, 260), np.float32)
    w1t = g("cls_w1").T                       # [256, 128]
    cblob[:, 0:128] = w1t[0:128]
    cblob[:, 128:256] = w1t[128:256]
    cblob[:, 256] = g("cls_b1")
    cblob[:, 257:259] = g("cls_w2").T
    cblob[0:2, 259] = g("cls_b2")

    common = {
        "embw": embw_p, "wblob": wblob, "fblob": fblob, "cblob": cblob,
    }

    x = g("x")[:, :T, :]              # causal truncation: only 32 steps matter
    in_maps = []
    for i in range(N_CORES):
        m = dict(common)
        m["x_local"] = np.ascontiguousarray(
            x[i * B_LOC:(i + 1) * B_LOC].reshape(TOK, 5))
        in_maps.append(m)
    return in_maps, a_vals


_PROGRAM_CACHE = {}


def kernel(**inputs) -> np.ndarray:
    in_maps, a_vals = _prep_host(inputs)
    nc = _PROGRAM_CACHE.get(a_vals)
    if nc is None:
        nc = _build_program(a_vals)
        _PROGRAM_CACHE[a_vals] = nc
    res = run_bass_kernel_spmd(nc, in_maps, core_ids=list(range(N_CORES)))
    out = np.zeros((BATCH, 2), np.float32)
    for i in range(N_CORES):
        out[i * B_LOC:(i + 1) * B_LOC] = np.asarray(res.results[i]["out"]).T
    return out
